# revision 28
# baseline (speedup 1.0000x reference)
"""Trainium2 Bass kernel for nn_EnhancedDepthwiseSeparableFFN.

Data-parallel over the batch: 8 samples -> 8 NeuronCores, one sample each.
Cross-core traffic: three tiny AllGathers for the BatchNorm batch statistics.

v2 layout/strategy (vs the f32r baseline):
  - all heavy matmuls in bf16 (PE 1 cyc/row, single-pass; f32r runs as
    LOW/HIGH pairs on hardware).
  - all weights baked into the NEFF as inline consts (loaded to HBM at
    model-load time); only x/xt ship per execution -> less per-run host
    transfer skew at the first collective, fewer in-kernel DMAs.
  - host pre-packs every SBUF layout so each load is one straight DMA.
  - AllGather combine via one strided tensor_reduce; fused BN affine;
  - srow/kw-chain via small const matmuls (CS validity-mask matrix);
  - ACT table prefetch dummies hide the 1.28us table reloads;
  - per-pixel channel max via PE transposes + DVE reduce (no gpsimd
    partition_all_reduce).
"""
import hashlib

import ml_dtypes
import numpy as np

import concourse.bass as bass
import concourse.bacc as bacc
import concourse.tile as tile
from concourse import mybir, bass_utils

F32 = mybir.dt.float32
F16 = mybir.dt.float16
AF = mybir.ActivationFunctionType
OP = mybir.AluOpType
BF = np.float16

D = 256          # model dim
C = 1024         # expanded channels
H = W = 32
HW = 1024
NCORES = 8
B = 8            # batch
EPS = 1e-5
CT = C // 128    # 8 channel tiles
HT = HW // 128   # 8 spatial tiles


# ---------------------------------------------------------------- host consts

def _interleave(a, k):
    """[k*128, n] -> [128, k*n]; dst[p, i*n:(i+1)*n] = a[i*128 + p]."""
    kk, n = a.shape
    assert kk == k * 128
    return np.ascontiguousarray(
        a.reshape(k, 128, n).transpose(1, 0, 2).reshape(128, k * n))


def _stencil_masks():
    """(128, 15*128) f32: columns = [d-1 q6..8 | d0 q0..q8 | d+1 q0..q2]."""
    k = np.arange(128)
    m = np.arange(128)
    r_in, w_in = k // 32, k % 32
    r_out, w_out = m // 32, m % 32
    dw = w_in[:, None] - w_out[None, :]
    tiles = []
    specs = [(-1, range(6, 9)), (0, range(9)), (1, range(3))]
    for delta, qs in specs:
        dh = r_in[:, None] - r_out[None, :] - 4 * delta
        for q in qs:
            dh_q, dw_q = q // 3 - 1, q % 3 - 1
            tiles.append(((dh == dh_q) & (dw == dw_q)).astype(np.float32))
    return np.concatenate(tiles, axis=1)


def _tap_valid():
    """(9, 1024): CS[q, n] = 1 if tap q is in-bounds at output pixel n."""
    n = np.arange(HW)
    r, w = n // W, n % W
    cs = np.zeros((9, HW), np.float32)
    for q in range(9):
        dh, dw = q // 3 - 1, q % 3 - 1
        cs[q] = ((r + dh >= 0) & (r + dh < H) & (w + dw >= 0) & (w + dw < W))
    return cs


def _spatial_bands(sw):
    """(128, 6*128) lhsT band tiles for the 7x7 conv."""
    k = np.arange(128)
    m = np.arange(128)
    r_in, w_in = k // 32, k % 32
    r_out, w_out = m // 32, m % 32
    dw = w_in[:, None] - w_out[None, :]
    wok = np.abs(dw) <= 3
    tiles = []
    for ch in range(2):
        for delta in (-1, 0, 1):
            dh = r_in[:, None] - r_out[None, :] - 4 * delta
            hok = np.abs(dh) <= 3
            t = np.zeros((128, 128), np.float32)
            ok = hok & wok
            t[ok] = sw[0, ch][(dh[ok] + 3, dw[ok] + 3)]
            tiles.append(t)
    return np.concatenate(tiles, axis=1)


def prep_weights(inputs):
    """Host-side packing of all weight tensors into on-device layouts."""
    f32 = lambda a: np.ascontiguousarray(np.asarray(a, np.float32))
    bf16 = lambda a: np.ascontiguousarray(np.asarray(a).astype(BF))
    w1 = f32(inputs["w1"])
    pw = f32(inputs["pw"])
    aw1 = f32(inputs["aw1"])
    aw2 = f32(inputs["aw2"])
    caw1 = f32(inputs["ca_w1"])
    caw2 = f32(inputs["ca_w2"])
    sw = f32(inputs["sw"])

    pwt = pw.T  # (C, D)
    paug = np.zeros((128, CT, 260), np.float32)
    paug[:, :, 0:D] = pwt.reshape(CT, 128, D).transpose(1, 0, 2)
    paug[:, :, D] = 1.0 / C
    W = {
        "w1t": bf16(_interleave(w1.T, 2)),                  # [128, 2*C]
        "aw1t": bf16(_interleave(aw1.T, 8)),                # [128, 8*128]
        "caw1t": bf16(_interleave(caw1.T, 8)),              # [128, 8*64]
        "caw2t": bf16(caw2.T),                              # [64, C]
        "aw2t": bf16(aw2.T),                                # [128, 9]
        "paug": bf16(paug.reshape(128, CT * 260)),          # [128, 8*260]
        "masks": bf16(_stencil_masks()),                    # [128, 15*128]
        "spb": bf16(_spatial_bands(sw)),                    # [128, 6*128]
        "cs": bf16(_tap_valid()),                           # [9, HW]
        "b1c": f32(inputs["b1"].reshape(CT, 128).T),
        "g1c": f32(inputs["g1"].reshape(CT, 128).T),
        "be1c": f32(inputs["be1"].reshape(CT, 128).T),
        "g2c": f32(inputs["g2"].reshape(CT, 128).T),
        "be2c": f32(inputs["be2"].reshape(CT, 128).T),
        "ab1c": f32(inputs["ab1"].reshape(1, 128).T),
        "ab2r": f32(inputs["ab2"].reshape(1, 9)),
        "g3r": f32(inputs["g3"].reshape(1, D)),
        "be3r": f32(inputs["be3"].reshape(1, D)),
        "sbc": np.full((128, 1), float(np.asarray(inputs["sb"]).reshape(())),
                       np.float32),
        "g3b": np.tile(f32(inputs["g3"]).reshape(1, D), (128, 1)),
        "be3b": np.tile(f32(inputs["be3"]).reshape(1, D), (128, 1)),
        "eye": bf16(np.eye(128, dtype=np.float32)),
        "eyef": np.eye(128, dtype=np.float32),
        "eye1b": bf16(np.ones((1, 1), np.float32)),
        "one1": np.ones((1, 1), np.float32),
        "onesr": np.ones((1, 128), np.float32),
        "onesrb": bf16(np.ones((1, 128), np.float32)),
        "onescb": bf16(np.ones((128, 1), np.float32)),
    }
    return W


# ---------------------------------------------------------------- the program

def build_program(W, n_cores=NCORES):
    nc = bacc.Bacc("TRN2", target_bir_lowering=False, debug=False,
                   num_devices=n_cores)

    # runtime inputs (per core, host-prepacked layouts)
    xt_in = nc.dram_tensor("xt", [128, 2 * HW], F16, kind="ExternalInput")
    xr_in = nc.dram_tensor("xres", [128, 8 * D], F16, kind="ExternalInput")
    out_d = nc.dram_tensor("out", [128, 8 * D], F32, kind="ExternalOutput")

    consts = {k: nc.inline_tensor(v, name="c_" + k) for k, v in W.items()}

    with tile.TileContext(nc) as tc:
        _body(nc, tc, consts, xt_in, xr_in, out_d, n_cores)
    nc.compile()
    return nc


def _body(nc, tc, CO, xt_in, xr_in, out_d, n_cores):
    nb = n_cores * HW  # BN normalizer

    with tc.tile_pool(name="sb", bufs=1) as sb, \
         tc.tile_pool(name="sb2", bufs=1) as sb2, \
         tc.tile_pool(name="psb", bufs=3, space="PSUM") as psB, \
         tc.tile_pool(name="psh", bufs=2, space="PSUM") as psH, \
         tc.tile_pool(name="dram", bufs=8, space="DRAM") as dram:

        # ---------------- loads: sync queue = compute-critical, gpsimd = rest
        def load(name, shape, dtype, ap_in, q):
            t = sb.tile(shape, dtype, tag=name)
            q.dma_start(t[:], ap_in)
            return t

        S, G = nc.sync, nc.gpsimd
        xt_sb = load("xt_sb", [128, 2 * HW], F16, xt_in.ap(), S)
        w1t_sb = load("w1t_sb", [128, 2 * C], F16, CO["w1t"].ap(), nc.scalar)
        xres = load("xres", [128, 8 * D], F16, xr_in.ap(), S)
        g3b = load("g3b", [128, D], F32, CO["g3b"].ap(), G)
        be3b = load("be3b", [128, D], F32, CO["be3b"].ap(), G)
        b1c = load("b1c", [128, CT], F32, CO["b1c"].ap(), S)
        g1c = load("g1c", [128, CT], F32, CO["g1c"].ap(), S)
        be1c = load("be1c", [128, CT], F32, CO["be1c"].ap(), S)
        ab1c = load("ab1c", [128, 1], F32, CO["ab1c"].ap(), S)
        ab2r = load("ab2r", [1, 9], F32, CO["ab2r"].ap(), S)
        tid = load("tid", [128, 128], F16, CO["eye"].ap(), S)
        tidf = load("tidf", [128, 128], F32, CO["eyef"].ap(), G)
        tid1b = load("tid1b", [1, 1], F16, CO["eye1b"].ap(), S)
        tid1 = load("tid1", [1, 1], F32, CO["one1"].ap(), S)
        tonesr = load("tonesr", [1, 128], F32, CO["onesr"].ap(), S)

        masks = load("masks", [128, 15 * 128], F16, CO["masks"].ap(), G)
        aw1t_sb = load("aw1t_sb", [128, 8 * 128], F16, CO["aw1t"].ap(), G)
        aw2t_sb = load("aw2t_sb", [128, 9], F16, CO["aw2t"].ap(), G)
        cs_sb = load("cs_sb", [9, HW], F16, CO["cs"].ap(), G)
        paug = load("paug", [128, CT * 260], F16, CO["paug"].ap(), G)
        caw1t_sb = load("caw1t_sb", [128, 8 * 64], F16, CO["caw1t"].ap(), G)
        caw2t_sb = load("caw2t_sb", [64, C], F16, CO["caw2t"].ap(), G)
        spb = load("spb", [128, 6 * 128], F16, CO["spb"].ap(), G)
        g2c = load("g2c", [128, CT], F32, CO["g2c"].ap(), G)
        be2c = load("be2c", [128, CT], F32, CO["be2c"].ap(), G)
        g3r = load("g3r", [1, D], F32, CO["g3r"].ap(), G)
        be3r = load("be3r", [1, D], F32, CO["be3r"].ap(), G)
        sbc = load("sbc", [128, 1], F32, CO["sbc"].ap(), G)
        tonesrb = load("tonesrb", [1, 128], F16, CO["onesrb"].ap(), G)
        tonescb = load("tonescb", [128, 1], F16, CO["onescb"].ap(), G)

        # big working tensors
        yg = sb2.tile([128, CT * HW], F16, tag="yg")       # gelu1 out (ch-major)
        yg_sp = sb2.tile([128, HT * C], F16, tag="ygsp")   # transposed
        g2o = sb2.tile([128, CT * HW], F16, tag="yg")      # gelu2 out (reuses yg)
        yca = sb2.tile([128, CT * HW], F16, tag="ygsp")    # ch-att out (reuses)
        stat1l = sb.tile([128, 16], F32, tag="stat1l")
        stat1g = sb.tile([128, 16], F32, tag="stat1g")
        stat2l = sb.tile([128, 16], F32, tag="stat2l")
        stat2g = sb.tile([128, 16], F32, tag="stat2g")
        scr_v = sb.tile([128, HW], F32, tag="scr_v")        # DVE STT scratch
        scr_g = sb.tile([128, HW], F32, tag="scr_g")        # GPSIMD STT scratch
        dmy = sb.tile([1, 1], F32, tag="dmy")
        dmyo = sb.tile([1, 1], F32, tag="dmyo")
        nc.vector.memset(dmy[:], 1.0)
        epsc = sb.tile([128, 1], F32, tag="epsc")
        nc.vector.memset(epsc[:], EPS)

        # ============================ PHASE 1: expand + gelu1 + stats1
        for m in range(CT):
            ps = psB.tile([128, HW], F32, tag="psb")
            for h in range(2):
                for k in range(2):
                    nc.tensor.matmul(
                        ps[:, h * 512:(h + 1) * 512],
                        w1t_sb[:, k * C + m * 128: k * C + (m + 1) * 128],
                        xt_sb[:, k * HW + h * 512: k * HW + (h + 1) * 512],
                        start=(k == 0), stop=(k == 1))
            nc.scalar.activation(
                yg[:, m * HW:(m + 1) * HW], ps[:],
                AF.Gelu, bias=b1c[:, m:m + 1], scale=1.0,
                accum_out=stat1l[:, m:m + 1])
            srcg = yg[:, m * HW:(m + 1) * HW]
            nc.vector.scalar_tensor_tensor(
                scr_v[:], srcg, 0.0, srcg, OP.bypass, OP.mult,
                accum_out=stat1l[:, 8 + m:9 + m])

        # ---- transposes yg -> yg_sp (overlap AG1)
        for ht in range(HT):
            for grp in range(2):
                pt = psH.tile([128, 512], F16, tag="psh")
                for j in range(4):
                    cb = grp * 4 + j
                    nc.tensor.transpose(
                        pt[:, j * 128:(j + 1) * 128],
                        yg[:, cb * HW + ht * 128: cb * HW + (ht + 1) * 128],
                        tid[:])
                dst = yg_sp[:, ht * C + grp * 512: ht * C + (grp + 1) * 512]
                if grp == 0:
                    nc.vector.tensor_copy(dst, pt[:])
                else:
                    nc.scalar.copy(dst, pt[:])

        # ============================ AG1 (BN1 batch stats)
        bb1i = dram.tile([128, 16], F32, tag="bb1i")
        bb1o = dram.tile([n_cores * 128, 16], F32, tag="bb1o")
        nc.gpsimd.dma_start(bb1i[:], stat1l[:])
        nc.gpsimd.collective_compute(
            "AllGather", OP.bypass, replica_groups=[list(range(n_cores))],
            ins=[bb1i.opt()], outs=[bb1o.opt()])
        gath1 = sb.tile([128, n_cores * 16], F32, tag="gath1")
        nc.gpsimd.dma_start(
            gath1[:].rearrange("p (r f) -> p r f", f=16),
            bb1o[:].rearrange("(r p) f -> p r f", p=128))
        # prefetch sqrt+exp tables during the collective (dep on stat1l so
        # the Tile scheduler places these in the AG1 stall window)
        nc.scalar.sqrt(dmyo[:], stat1l[0:1, 0:1])
        nc.scalar.activation(dmyo[:], stat1l[0:1, 0:1], AF.Exp, bias=0.0,
                             scale=1.0)
        # PE p-state warmers: dummy transposes gated on the gathered stats,
        # so the PE is at speed when the stencil starts
        wps = psH.tile([128, 128], F32, tag="psh")
        for _ in range(20):
            nc.tensor.matmul(wps[:], gath1[:, 0:128], tidf[:],
                             is_transpose=True, skip_group_check=True)
        # combine 8 replicas in one strided reduce
        nc.vector.tensor_reduce(
            stat1g[:], gath1[:].rearrange("p (r f) -> p f r", f=16),
            mybir.AxisListType.X, OP.add)

        # ============================ BN1 affine + kw + L build
        def bn_affine(statg, gcol, becol, tagp):
            """-> (a, bn) per-channel scale/shift columns (128, CT), f32."""
            msq = sb.tile([128, 16], F32, tag=tagp + "m")
            var = sb.tile([128, CT], F32, tag=tagp + "v")
            sd = sb.tile([128, CT], F32, tag=tagp + "s")
            a = sb.tile([128, CT], F32, tag=tagp + "a")
            bn = sb.tile([128, CT], F32, tag=tagp + "b")
            nc.vector.tensor_scalar_mul(msq[:], statg[:], 1.0 / nb)
            mns, ex2 = msq[:, 0:8], msq[:, 8:16]
            nc.vector.tensor_tensor(var[:], mns, mns, OP.mult)
            nc.vector.tensor_tensor(var[:], ex2, var[:], OP.subtract)
            nc.scalar.activation(sd[:], var[:], AF.Sqrt, bias=epsc[:],
                                 scale=1.0)
            nc.vector.reciprocal(a[:], sd[:])
            nc.vector.tensor_tensor(a[:], gcol[:], a[:], OP.mult)
            nc.vector.tensor_tensor(bn[:], mns, a[:], OP.mult)
            nc.vector.tensor_tensor(bn[:], becol[:], bn[:], OP.subtract)
            return a, bn, msq

        a1, b1n, _ = bn_affine(stat1g, g1c, be1c, "s1")
        # b' = b1n / a1, as bf16 rows [CT, 128] (rank-1 lhsT slices)
        inva1 = sb.tile([128, CT], F32, tag="inva1")
        bpre = sb.tile([128, CT], F32, tag="bpre")
        nc.vector.reciprocal(inva1[:], a1[:])
        nc.vector.tensor_tensor(bpre[:], b1n[:], inva1[:], OP.mult)
        psb1 = psH.tile([CT, 128], F32, tag="psh")
        nc.tensor.transpose(psb1[:], bpre[:], tidf[:])
        b1pr8 = sb.tile([CT, 128], F16, tag="b1pr8")
        nc.vector.tensor_copy(b1pr8[:], psb1[:])
        # flatten [CT,128] -> [1, C] row via a DRAM bounce (engines cannot
        # move data across partitions; PE lhsT base partition must be 0)
        bpd = dram.tile([CT, 128], F16, tag="bpd")
        nc.gpsimd.dma_start(bpd[:], b1pr8[:])
        b1rowb = sb.tile([1, C], F16, tag="b1rowb")
        nc.gpsimd.dma_start(b1rowb[:], bpd[:].rearrange("c p -> (c p)")[None, :])

        # gap (local, normalized) -> kw
        gapnb = sb.tile([128, CT], F16, tag="gapnb")
        gapn = sb.tile([128, CT], F32, tag="gapn")
        nc.vector.scalar_tensor_tensor(gapn[:], stat1l[:, 0:8], 1.0 / HW,
                                       a1[:], OP.mult, OP.mult)
        nc.vector.tensor_tensor(gapnb[:], gapn[:], b1n[:], OP.add)

        ph1 = psH.tile([128, 1], F32, tag="psh")
        for k in range(CT):
            nc.tensor.matmul(ph1[:], aw1t_sb[:, k * 128:(k + 1) * 128],
                             gapnb[:, k:k + 1], start=(k == 0), stop=(k == 7))
        h1 = sb.tile([128, 1], F16, tag="h1")
        nc.scalar.activation(h1[:], ph1[:], AF.Relu, bias=ab1c[:], scale=1.0)
        ps9 = psH.tile([1, 9], F32, tag="psh")
        nc.tensor.matmul(ps9[:], h1[:], aw2t_sb[:], start=True, stop=True)
        v9 = sb.tile([1, 9], F32, tag="v9")
        nc.vector.tensor_tensor(v9[:], ps9[:], ab2r[:], OP.add)
        nmx = sb.tile([1, 1], F32, tag="nmx")
        nc.vector.tensor_reduce(nmx[:], v9[:], mybir.AxisListType.X, OP.max,
                                negate=True)
        e9 = sb.tile([1, 9], F32, tag="e9")
        se = sb.tile([1, 1], F32, tag="se")
        nc.scalar.activation(e9[:], v9[:], AF.Exp, bias=nmx[:], scale=1.0,
                             accum_out=se[:])
        rse = sb.tile([1, 1], F32, tag="rse")
        nc.vector.reciprocal(rse[:], se[:])
        kw9 = sb.tile([1, 9], F32, tag="kw9")
        nc.vector.tensor_scalar(kw9[:], e9[:], rse[:], None, OP.mult)
        # prefetch gelu table for phase 4 while the L build runs
        nc.scalar.activation(dmyo[:], kw9[0:1, 0:1], AF.Gelu, bias=0.0,
                             scale=1.0)

        # kw broadcast to all partitions (f32, for the DVE L build)
        pskb = psH.tile([128, 9], F32, tag="psh")
        nc.tensor.matmul(pskb[:], tonesr[:], kw9[:], start=True, stop=True)
        kwb = sb.tile([128, 9], F32, tag="kwb")
        nc.scalar.copy(kwb[:], pskb[:])
        # kw as bf16 column [9, 1] for the srow matmul
        pskc = psH.tile([9, 1], F32, tag="psh")
        nc.tensor.transpose(pskc[:], kw9[:], tid1[:])
        kwcb = sb.tile([9, 1], F16, tag="kwcb")
        nc.scalar.copy(kwcb[:], pskc[:])
        # srow = kw^T @ CS (tap-validity matrix), bf16
        srowb = sb.tile([1, HW], F16, tag="srowb")
        for h in range(2):
            psu = psH.tile([1, 512], F32, tag="psh")
            nc.tensor.matmul(psu[:], kwcb[:], cs_sb[:, h * 512:(h + 1) * 512],
                             start=True, stop=True)
            nc.scalar.copy(srowb[:, h * 512:(h + 1) * 512], psu[:])

        # L band tiles [L(-1) | L(0) | L(+1)]: ACT does the 6 side-block
        # products, DVE chains the 9 center STTs + 4 adds; bf16 at the end.
        L = sb.tile([128, 3 * 128], F32, tag="L")
        Lb = sb.tile([128, 3 * 128], F16, tag="Lb")
        prods = sb.tile([128, 6 * 128], F32, tag="prods")
        side_qs = [(0, 6), (1, 7), (2, 8), (12, 0), (13, 1), (14, 2)]
        for i, (blk, q) in enumerate(side_qs):
            nc.scalar.activation(prods[:, i * 128:(i + 1) * 128],
                                 masks[:, blk * 128:(blk + 1) * 128],
                                 AF.Copy, bias=0.0, scale=kwb[:, q:q + 1])
        nc.vector.tensor_scalar(L[:, 0:128], masks[:, 3 * 128:4 * 128],
                                kwb[:, 0:1], None, OP.mult)
        # Ld0 center chain: L[:,0:128] holds running sum of q0..q8 products
        for q in range(1, 8):
            nc.vector.scalar_tensor_tensor(
                L[:, 0:128], masks[:, (3 + q) * 128:(4 + q) * 128],
                kwb[:, q:q + 1], L[:, 0:128], OP.mult, OP.add)
        nc.vector.scalar_tensor_tensor(
            Lb[:, 128:256], masks[:, 11 * 128:12 * 128],
            kwb[:, 8:9], L[:, 0:128], OP.mult, OP.add)
        # Ldm = p0+p1+p2, Ldp = p3+p4+p5 (f32 partials from ACT, bf16 out)
        nc.vector.tensor_tensor(L[:, 128:256], prods[:, 0:128],
                                prods[:, 128:256], OP.add)
        nc.vector.tensor_tensor(Lb[:, 0:128], L[:, 128:256],
                                prods[:, 256:384], OP.add)
        nc.vector.tensor_tensor(L[:, 256:384], prods[:, 384:512],
                                prods[:, 512:640], OP.add)
        nc.vector.tensor_tensor(Lb[:, 256:384], L[:, 256:384],
                                prods[:, 640:768], OP.add)

        # ============================ PHASE 4: stencil + gelu2 + stats2
        for c in range(CT):
            psz = psB.tile([128, HW], F32, tag="psb")
            for h in range(2):
                nc.tensor.matmul(psz[:, h * 512:(h + 1) * 512],
                                 b1rowb[:, c * 128:(c + 1) * 128],
                                 srowb[:, h * 512:(h + 1) * 512],
                                 start=True, stop=False)
            for t_in in range(HT):
                lo = max(0, (t_in - 1) * 128)
                hi = min(HW, (t_in + 2) * 128)
                roff = 128 + (lo - t_in * 128)
                if lo < 512 < hi:
                    pieces = [(lo, 512), (512, hi)]
                else:
                    pieces = [(lo, hi)]
                for (a, b) in pieces:
                    ra = roff + (a - lo)
                    last_bank0 = (a < 512) and (t_in == 4)
                    last_bank1 = (a >= 512) and (t_in == 7)
                    nc.tensor.matmul(
                        psz[:, a:b],
                        yg_sp[:, t_in * C + c * 128: t_in * C + (c + 1) * 128],
                        Lb[:, ra:ra + (b - a)],
                        start=False, stop=(last_bank0 or last_bank1))
            nc.scalar.activation(
                g2o[:, c * HW:(c + 1) * HW], psz[:], AF.Gelu,
                bias=0.0, scale=a1[:, c:c + 1],
                accum_out=stat2l[:, c:c + 1])
            srcg2 = g2o[:, c * HW:(c + 1) * HW]
            nc.vector.scalar_tensor_tensor(
                scr_v[:], srcg2, 0.0, srcg2, OP.bypass, OP.mult,
                accum_out=stat2l[:, 8 + c:9 + c])

        # ============================ AG2 (BN2 batch stats)
        bb2i = dram.tile([128, 16], F32, tag="bb2i")
        bb2o = dram.tile([n_cores * 128, 16], F32, tag="bb2o")
        nc.gpsimd.dma_start(bb2i[:], stat2l[:])
        nc.gpsimd.collective_compute(
            "AllGather", OP.bypass, replica_groups=[list(range(n_cores))],
            ins=[bb2i.opt()], outs=[bb2o.opt()])
        gath2 = sb.tile([128, n_cores * 16], F32, tag="gath2")
        nc.gpsimd.dma_start(
            gath2[:].rearrange("p (r f) -> p r f", f=16),
            bb2o[:].rearrange("(r p) f -> p r f", p=128))
        nc.scalar.sqrt(dmyo[:], stat2l[0:1, 0:1])  # prefetch sqrt table
        wps2 = psH.tile([128, 128], F32, tag="psh")
        for _ in range(20):
            nc.tensor.matmul(wps2[:], gath2[:, 0:128], tidf[:],
                             is_transpose=True, skip_group_check=True)
        # overlap: per-channel max over HW of g2o (local)
        mxc = sb.tile([128, CT], F32, tag="mxc")
        for c in range(CT):
            nc.vector.tensor_reduce(mxc[:, c:c + 1],
                                    g2o[:, c * HW:(c + 1) * HW],
                                    mybir.AxisListType.X, OP.max)
        nc.vector.tensor_reduce(
            stat2g[:], gath2[:].rearrange("p (r f) -> p f r", f=16),
            mybir.AxisListType.X, OP.add)

        # ============================ BN2 + channel attention
        a2, b2n, _ = bn_affine(stat2g, g2c, be2c, "s2")
        amxb = sb.tile([128, 2 * CT], F16, tag="amxb")
        amx0 = sb.tile([128, CT], F32, tag="amx0")
        nc.vector.scalar_tensor_tensor(amx0[:], stat2l[:, 0:8], 1.0 / HW,
                                       a2[:], OP.mult, OP.mult)
        nc.vector.tensor_tensor(amxb[:, 0:8], amx0[:], b2n[:], OP.add)
        nc.vector.tensor_tensor(amx0[:], mxc[:], a2[:], OP.mult)
        nc.vector.tensor_tensor(amxb[:, 8:16], amx0[:], b2n[:], OP.add)

        psf = psH.tile([64, 2], F32, tag="psh")
        for k in range(CT):
            nc.tensor.matmul(psf[:], caw1t_sb[:, k * 64:(k + 1) * 64],
                             amxb[:, k:k + 9:8], start=(k == 0), stop=(k == 7))
        hp = sb.tile([64, 2], F16, tag="hp")
        nc.scalar.activation(hp[:], psf[:], AF.Relu, bias=0.0, scale=1.0)
        hsum = sb.tile([64, 1], F16, tag="hsum")
        nc.vector.tensor_tensor(hsum[:], hp[:, 0:1], hp[:, 1:2], OP.add)
        # prefetch sigmoid table while the FC2 matmuls run
        nc.scalar.activation(dmyo[:], amxb[0:1, 0:1], AF.Sigmoid, bias=0.0,
                             scale=1.0)

        pss = psH.tile([128, CT], F32, tag="psh")
        for c in range(CT):
            nc.tensor.matmul(pss[:, c:c + 1], caw2t_sb[:, c * 128:(c + 1) * 128],
                             hsum[:], start=True, stop=True)
        scol = sb.tile([128, CT], F32, tag="scol")
        nc.scalar.activation(scol[:], pss[:], AF.Sigmoid, bias=0.0, scale=1.0)

        sprime = sb.tile([128, CT], F32, tag="sprime")
        b2s = sb.tile([128, CT], F32, tag="b2s")
        nc.vector.tensor_tensor(sprime[:], scol[:], a2[:], OP.mult)
        nc.vector.tensor_tensor(b2s[:], scol[:], b2n[:], OP.mult)
        b2sb = sb.tile([128, CT], F16, tag="b2sb")
        nc.vector.tensor_copy(b2sb[:], b2s[:])

        # y_ca (for the channel-max / channel-mean of spatial attention)
        for c in range(CT):
            nc.vector.tensor_scalar(yca[:, c * HW:(c + 1) * HW],
                                    g2o[:, c * HW:(c + 1) * HW],
                                    sprime[:, c:c + 1], b2s[:, c:c + 1],
                                    OP.mult, OP.add)

        # scaled projection weights
        pws = sb2.tile([128, CT * 260], F16, tag="pws")
        for c in range(CT):
            nc.vector.tensor_scalar(pws[:, c * 260:c * 260 + 258],
                                    paug[:, c * 260:c * 260 + 258],
                                    sprime[:, c:c + 1], None, OP.mult)
        # t2 row (rank-1 bias of the projection)
        pst2 = psH.tile([1, 258], F32, tag="psh")
        for c in range(CT):
            nc.tensor.matmul(pst2[:], b2sb[:, c:c + 1],
                             paug[:, c * 260:c * 260 + 258],
                             start=(c == 0), stop=(c == 7))
        u2row = sb.tile([1, 258], F16, tag="u2row")
        nc.scalar.copy(u2row[:], pst2[:])

        # projection -> proj_sb (spatial-major (hw, d)), bf16
        proj_sb = sb2.tile([128, 8 * D], F16, tag="proj_sb")
        avgpx = sb.tile([128, HT], F16, tag="avgpx")
        for mt in range(HT):
            psp = psH.tile([128, 258], F32, tag="psh")
            for c in range(CT):
                nc.tensor.matmul(psp[:],
                                 g2o[:, c * HW + mt * 128: c * HW + (mt + 1) * 128],
                                 pws[:, c * 260:c * 260 + 258],
                                 start=(c == 0), stop=False)
            nc.tensor.matmul(psp[:], tonesrb[:], u2row[:], start=False,
                             stop=True)
            dst = proj_sb[:, mt * D:(mt + 1) * D]
            if mt % 2 == 0:
                nc.vector.tensor_copy(dst, psp[:, 0:D])
            else:
                nc.scalar.copy(dst, psp[:, 0:D])
            nc.vector.tensor_copy(avgpx[:, mt:mt + 1], psp[:, D:D + 1])

        # channel max per pixel: tree over c-tiles (destroys yca), then
        # PE-transpose each 128-pixel block and DVE-reduce over channels
        for i in range(4):
            nc.vector.tensor_tensor(yca[:, (2 * i) * HW:(2 * i + 1) * HW],
                                    yca[:, (2 * i) * HW:(2 * i + 1) * HW],
                                    yca[:, (2 * i + 1) * HW:(2 * i + 2) * HW],
                                    OP.max)
        nc.vector.tensor_tensor(yca[:, 0:HW], yca[:, 0:HW],
                                yca[:, 2 * HW:3 * HW], OP.max)
        nc.vector.tensor_tensor(yca[:, 4 * HW:5 * HW], yca[:, 4 * HW:5 * HW],
                                yca[:, 6 * HW:7 * HW], OP.max)
        nc.vector.tensor_tensor(yca[:, 0:HW], yca[:, 0:HW],
                                yca[:, 4 * HW:5 * HW], OP.max)
        mxpx = sb.tile([128, HT], F16, tag="mxpx")
        for t in range(HT):
            pmx = psH.tile([128, 128], F16, tag="psh")
            nc.tensor.transpose(pmx[:], yca[:, t * 128:(t + 1) * 128], tid[:])
            nc.vector.tensor_reduce(mxpx[:, t:t + 1], pmx[:],
                                    mybir.AxisListType.X, OP.max)

        # spatial 7x7 conv as 6 shifted-column matmuls, one sigmoid
        pssp = psH.tile([128, HT], F32, tag="psh")
        mmspecs = []
        for ch, srccol in ((0, avgpx), (1, mxpx)):
            mmspecs.append((ch * 3 + 1, slice(0, 8), srccol[:, 0:8]))
            mmspecs.append((ch * 3 + 2, slice(1, 8), srccol[:, 0:7]))
            mmspecs.append((ch * 3 + 0, slice(0, 7), srccol[:, 1:8]))
        for i, (bi, osl, rhs) in enumerate(mmspecs):
            nc.tensor.matmul(pssp[:, osl], spb[:, bi * 128:(bi + 1) * 128],
                             rhs, start=(i == 0), stop=(i == len(mmspecs) - 1))
        spcol = sb.tile([128, HT], F32, tag="spcol")
        nc.scalar.activation(spcol[:], pssp[:], AF.Sigmoid, bias=sbc[:],
                             scale=1.0)
        spcolb = sb.tile([128, HT], F16, tag="spcolb")
        nc.vector.tensor_copy(spcolb[:], spcol[:])

        # spp = proj * sp (spatial scale, per-partition)
        spp = sb2.tile([128, 8 * D], F16, tag="spp")
        for mt in range(HT):
            nc.vector.tensor_scalar(spp[:, mt * D:(mt + 1) * D],
                                    proj_sb[:, mt * D:(mt + 1) * D],
                                    spcol[:, mt:mt + 1], None, OP.mult)

        # BN3 stats: sum(sp*proj) and sum((sp*proj)^2) over hw
        pst3a = psH.tile([1, D], F32, tag="psh")
        for mt in range(HT):
            nc.tensor.matmul(pst3a[:], spcolb[:, mt:mt + 1],
                             proj_sb[:, mt * D:(mt + 1) * D],
                             start=(mt == 0), stop=(mt == 7))
        pst3b = psH.tile([1, D], F32, tag="psh")
        sqs = sb.tile([128, 2 * D], F16, tag="sqs")
        for mt in range(HT):
            half = (mt % 2) * D
            nc.scalar.square(sqs[:, half:half + D], spp[:, mt * D:(mt + 1) * D])
            nc.tensor.matmul(pst3b[:], tonescb[:], sqs[:, half:half + D],
                             start=(mt == 0), stop=(mt == 7))
        stat3l = sb.tile([1, 2 * D], F32, tag="stat3l")
        nc.scalar.copy(stat3l[:, 0:D], pst3a[:])
        nc.vector.tensor_copy(stat3l[:, D:2 * D], pst3b[:])

        # ============================ AG3 (BN3 batch stats)
        bb3i = dram.tile([1, 2 * D], F32, tag="bb3i")
        bb3o = dram.tile([n_cores, 2 * D], F32, tag="bb3o")
        nc.gpsimd.dma_start(bb3i[:], stat3l[:])
        nc.gpsimd.collective_compute(
            "AllGather", OP.bypass, replica_groups=[list(range(n_cores))],
            ins=[bb3i.opt()], outs=[bb3o.opt()])
        gath3 = sb.tile([1, n_cores * 2 * D], F32, tag="gath3")
        nc.gpsimd.dma_start(gath3[:],
                            bb3o[:].rearrange("r f -> (r f)")[None, :])
        nc.scalar.sqrt(dmyo[:], stat3l[0:1, 0:1])  # prefetch sqrt table
        wps3 = psH.tile([128, 128], F32, tag="psh")
        for _ in range(12):
            nc.tensor.matmul(wps3[:], gath3[:, 0:128].rearrange("o f -> o f"),
                             tidf[0:1, :], is_transpose=True,
                             skip_group_check=True)
        # combine via a contiguous add tree (strided row reduce is slow)
        stat3g = sb.tile([1, 2 * D], F32, tag="stat3g")
        g3v = gath3[:]
        W2 = 2 * D
        for r in (4, 2, 1):
            for i in range(r):
                nc.vector.tensor_tensor(
                    g3v[:, i * W2:(i + 1) * W2] if r > 1 else stat3g[:],
                    g3v[:, (2 * i) * W2:(2 * i + 1) * W2],
                    g3v[:, (2 * i + 1) * W2:(2 * i + 2) * W2], OP.add)

        # BN3 affine broadcast-first: two row ops, then everything at
        # [128, D] full-lane width (pb cancels against the mean)
        msq3 = sb.tile([1, 2 * D], F32, tag="msq3")
        nc.vector.tensor_scalar_mul(msq3[:], stat3g[:], 1.0 / nb)
        m3b = sb.tile([128, D], F32, tag="m3b")
        psx = psH.tile([128, 2 * D], F32, tag="psh")
        nc.tensor.matmul(psx[:, 0:D], tonesr[:], msq3[:, 0:D],
                         start=True, stop=True)
        nc.tensor.matmul(psx[:, D:2 * D], tonesr[:], msq3[:, D:2 * D],
                         start=True, stop=True)
        nc.vector.tensor_copy(m3b[:], psx[:, 0:D])
        vb = sb.tile([128, D], F32, tag="vb")
        nc.vector.tensor_tensor(vb[:], m3b[:], m3b[:], OP.mult)
        nc.vector.tensor_tensor(vb[:], psx[:, D:2 * D], vb[:], OP.subtract)
        nc.scalar.activation(vb[:], vb[:], AF.Sqrt, bias=epsc[:], scale=1.0)
        a3b = sb.tile([128, D], F16, tag="a3b")
        c3b = sb.tile([128, D], F16, tag="c3b")
        rb = sb.tile([128, D], F32, tag="rb")
        nc.vector.reciprocal(rb[:], vb[:])
        nc.vector.tensor_tensor(a3b[:], g3b[:], rb[:], OP.mult)
        nc.vector.tensor_tensor(rb[:], m3b[:], rb[:], OP.mult)
        nc.vector.tensor_tensor(rb[:], g3b[:], rb[:], OP.mult)
        nc.vector.tensor_tensor(c3b[:], be3b[:], rb[:], OP.subtract)

        # final: out = (xres + c3) + spp*a3
        # gpsimd computes xres+c3 per block; DVE does the mult and final add
        xc = sb2.tile([128, 8 * D], F32, tag="xc")
        out_sb = sb2.tile([128, 8 * D], F32, tag="outsb")
        tmp = sb.tile([128, D], F16, tag="ftmp")
        for mt in range(HT):
            sl = slice(mt * D, (mt + 1) * D)
            nc.gpsimd.tensor_tensor(xc[:, sl], xres[:, sl], c3b[:], OP.add)
            nc.vector.tensor_tensor(tmp[:], spp[:, sl], a3b[:], OP.mult)
            nc.vector.tensor_tensor(out_sb[:, sl], xc[:, sl], tmp[:], OP.add)
            q = nc.sync if mt % 2 == 0 else nc.scalar
            q.dma_start(out_d.ap()[:, mt * D:(mt + 1) * D], out_sb[:, sl])


# ---------------------------------------------------------------- host driver

def shard_inputs(inputs):
    x = np.ascontiguousarray(np.asarray(inputs["x"], np.float32))
    in_maps = []
    for i in range(NCORES):
        in_maps.append({
            "xt": _interleave(x[i].T, 2).astype(BF),
            "xres": _interleave(x[i], 8).astype(BF),
        })
    return in_maps


_CACHE = {}


def get_program(inputs, n_cores=NCORES):
    W = prep_weights(inputs)
    h = hashlib.sha256()
    for k in sorted(W):
        h.update(k.encode())
        h.update(np.ascontiguousarray(W[k]).tobytes())
    key = (n_cores, h.hexdigest())
    if key not in _CACHE:
        _CACHE[key] = build_program(W, n_cores=n_cores)
    return _CACHE[key]


def run(inputs, trace=False):
    nc = get_program(inputs)
    in_maps = shard_inputs(inputs)
    r = bass_utils.run_bass_kernel_spmd(
        nc, in_maps, core_ids=list(range(NCORES)), trace=trace)
    out = np.stack(
        [r.results[i]["out"].reshape(128, 8, D).transpose(1, 0, 2)
         .reshape(HW, D) for i in range(NCORES)], axis=0)
    return np.ascontiguousarray(out.astype(np.float32)), r


def kernel(**inputs) -> np.ndarray:
    out, _ = run(inputs, trace=False)
    return out


# revision 29
# speedup vs baseline: 1.1029x; 1.1029x over previous
"""Trainium2 Bass kernel for nn_EnhancedDepthwiseSeparableFFN.

Data-parallel over the batch: 8 samples -> 8 NeuronCores, one sample each.
Cross-core traffic: three tiny AllGathers for the BatchNorm batch statistics.

v2 layout/strategy (vs the f32r baseline):
  - all heavy matmuls in bf16 (PE 1 cyc/row, single-pass; f32r runs as
    LOW/HIGH pairs on hardware).
  - all weights baked into the NEFF as inline consts (loaded to HBM at
    model-load time); only x/xt ship per execution -> less per-run host
    transfer skew at the first collective, fewer in-kernel DMAs.
  - host pre-packs every SBUF layout so each load is one straight DMA.
  - AllGather combine via one strided tensor_reduce; fused BN affine;
  - srow/kw-chain via small const matmuls (CS validity-mask matrix);
  - ACT table prefetch dummies hide the 1.28us table reloads;
  - per-pixel channel max via PE transposes + DVE reduce (no gpsimd
    partition_all_reduce).
"""
import hashlib

import ml_dtypes
import numpy as np

import concourse.bass as bass
import concourse.bacc as bacc
import concourse.tile as tile
from concourse import mybir, bass_utils

F32 = mybir.dt.float32
F16 = mybir.dt.float16
AF = mybir.ActivationFunctionType
OP = mybir.AluOpType
BF = np.float16

D = 256          # model dim
C = 1024         # expanded channels
H = W = 32
HW = 1024
NCORES = 8
B = 8            # batch
EPS = 1e-5
CT = C // 128    # 8 channel tiles
HT = HW // 128   # 8 spatial tiles


# ---------------------------------------------------------------- host consts

def _interleave(a, k):
    """[k*128, n] -> [128, k*n]; dst[p, i*n:(i+1)*n] = a[i*128 + p]."""
    kk, n = a.shape
    assert kk == k * 128
    return np.ascontiguousarray(
        a.reshape(k, 128, n).transpose(1, 0, 2).reshape(128, k * n))


def _stencil_masks():
    """(128, 15*128) f32: columns = [d-1 q6..8 | d0 q0..q8 | d+1 q0..q2]."""
    k = np.arange(128)
    m = np.arange(128)
    r_in, w_in = k // 32, k % 32
    r_out, w_out = m // 32, m % 32
    dw = w_in[:, None] - w_out[None, :]
    tiles = []
    specs = [(-1, range(6, 9)), (0, range(9)), (1, range(3))]
    for delta, qs in specs:
        dh = r_in[:, None] - r_out[None, :] - 4 * delta
        for q in qs:
            dh_q, dw_q = q // 3 - 1, q % 3 - 1
            tiles.append(((dh == dh_q) & (dw == dw_q)).astype(np.float32))
    return np.concatenate(tiles, axis=1)


def _tap_valid():
    """(9, 1024): CS[q, n] = 1 if tap q is in-bounds at output pixel n."""
    n = np.arange(HW)
    r, w = n // W, n % W
    cs = np.zeros((9, HW), np.float32)
    for q in range(9):
        dh, dw = q // 3 - 1, q % 3 - 1
        cs[q] = ((r + dh >= 0) & (r + dh < H) & (w + dw >= 0) & (w + dw < W))
    return cs


def _spatial_bands(sw):
    """(128, 6*128) lhsT band tiles for the 7x7 conv."""
    k = np.arange(128)
    m = np.arange(128)
    r_in, w_in = k // 32, k % 32
    r_out, w_out = m // 32, m % 32
    dw = w_in[:, None] - w_out[None, :]
    wok = np.abs(dw) <= 3
    tiles = []
    for ch in range(2):
        for delta in (-1, 0, 1):
            dh = r_in[:, None] - r_out[None, :] - 4 * delta
            hok = np.abs(dh) <= 3
            t = np.zeros((128, 128), np.float32)
            ok = hok & wok
            t[ok] = sw[0, ch][(dh[ok] + 3, dw[ok] + 3)]
            tiles.append(t)
    return np.concatenate(tiles, axis=1)


def prep_weights(inputs):
    """Host-side packing of all weight tensors into on-device layouts."""
    f32 = lambda a: np.ascontiguousarray(np.asarray(a, np.float32))
    bf16 = lambda a: np.ascontiguousarray(np.asarray(a).astype(BF))
    w1 = f32(inputs["w1"])
    pw = f32(inputs["pw"])
    aw1 = f32(inputs["aw1"])
    aw2 = f32(inputs["aw2"])
    caw1 = f32(inputs["ca_w1"])
    caw2 = f32(inputs["ca_w2"])
    sw = f32(inputs["sw"])

    pwt = pw.T  # (C, D)
    paug = np.zeros((128, CT, 260), np.float32)
    paug[:, :, 0:D] = pwt.reshape(CT, 128, D).transpose(1, 0, 2)
    paug[:, :, D] = 1.0 / C
    W = {
        "w1t": bf16(_interleave(w1.T, 2)),                  # [128, 2*C]
        "aw1t": bf16(_interleave(aw1.T, 8)),                # [128, 8*128]
        "caw1t": bf16(_interleave(caw1.T, 8)),              # [128, 8*64]
        "caw2t": bf16(caw2.T),                              # [64, C]
        "aw2t": bf16(aw2.T),                                # [128, 9]
        "paug": bf16(paug.reshape(128, CT * 260)),          # [128, 8*260]
        "masks": bf16(_stencil_masks()),                    # [128, 15*128]
        "spb": bf16(_spatial_bands(sw)),                    # [128, 6*128]
        "cs": bf16(_tap_valid()),                           # [9, HW]
        "b1c": f32(inputs["b1"].reshape(CT, 128).T),
        "g1c": f32(inputs["g1"].reshape(CT, 128).T),
        "be1c": f32(inputs["be1"].reshape(CT, 128).T),
        "g2c": f32(inputs["g2"].reshape(CT, 128).T),
        "be2c": f32(inputs["be2"].reshape(CT, 128).T),
        "ab1c": f32(inputs["ab1"].reshape(1, 128).T),
        "ab2r": f32(inputs["ab2"].reshape(1, 9)),
        "g3r": f32(inputs["g3"].reshape(1, D)),
        "be3r": f32(inputs["be3"].reshape(1, D)),
        "sbc": np.full((128, 1), float(np.asarray(inputs["sb"]).reshape(())),
                       np.float32),
        "g3b": np.tile(f32(inputs["g3"]).reshape(1, D), (128, 1)),
        "be3b": np.tile(f32(inputs["be3"]).reshape(1, D), (128, 1)),
        "eye": bf16(np.eye(128, dtype=np.float32)),
        "eyef": np.eye(128, dtype=np.float32),
        "eye1b": bf16(np.ones((1, 1), np.float32)),
        "one1": np.ones((1, 1), np.float32),
        "onesr": np.ones((1, 128), np.float32),
        "onesrb": bf16(np.ones((1, 128), np.float32)),
        "onescb": bf16(np.ones((128, 1), np.float32)),
        "ones8": np.ones((8, 1), np.float32),
    }
    return W


# ---------------------------------------------------------------- the program

def build_program(W, n_cores=NCORES):
    nc = bacc.Bacc("TRN2", target_bir_lowering=False, debug=False,
                   num_devices=n_cores)

    # runtime inputs (per core, host-prepacked layouts)
    xt_in = nc.dram_tensor("xt", [128, 2 * HW], F16, kind="ExternalInput")
    xr_in = nc.dram_tensor("xres", [128, 8 * D], F16, kind="ExternalInput")
    out_d = nc.dram_tensor("out", [128, 8 * D], F32, kind="ExternalOutput")

    consts = {k: nc.inline_tensor(v, name="c_" + k) for k, v in W.items()}

    with tile.TileContext(nc) as tc:
        _body(nc, tc, consts, xt_in, xr_in, out_d, n_cores)
    nc.compile()
    return nc


def _body(nc, tc, CO, xt_in, xr_in, out_d, n_cores):
    nb = n_cores * HW  # BN normalizer

    with tc.tile_pool(name="sb", bufs=1) as sb, \
         tc.tile_pool(name="sb2", bufs=1) as sb2, \
         tc.tile_pool(name="psb", bufs=3, space="PSUM") as psB, \
         tc.tile_pool(name="psh", bufs=2, space="PSUM") as psH, \
         tc.tile_pool(name="dram", bufs=8, space="DRAM") as dram:

        # ---------------- loads: sync queue = compute-critical, gpsimd = rest
        def load(name, shape, dtype, ap_in, q):
            t = sb.tile(shape, dtype, tag=name)
            q.dma_start(t[:], ap_in)
            return t

        S, G = nc.sync, nc.gpsimd
        xt_sb = load("xt_sb", [128, 2 * HW], F16, xt_in.ap(), S)
        w1t_sb = load("w1t_sb", [128, 2 * C], F16, CO["w1t"].ap(), nc.scalar)
        xres = load("xres", [128, 8 * D], F16, xr_in.ap(), S)
        g3b = load("g3b", [128, D], F32, CO["g3b"].ap(), G)
        be3b = load("be3b", [128, D], F32, CO["be3b"].ap(), G)
        b1c = load("b1c", [128, CT], F32, CO["b1c"].ap(), S)
        g1c = load("g1c", [128, CT], F32, CO["g1c"].ap(), S)
        be1c = load("be1c", [128, CT], F32, CO["be1c"].ap(), S)
        ab1c = load("ab1c", [128, 1], F32, CO["ab1c"].ap(), S)
        ab2r = load("ab2r", [1, 9], F32, CO["ab2r"].ap(), S)
        tid = load("tid", [128, 128], F16, CO["eye"].ap(), S)
        tidf = load("tidf", [128, 128], F32, CO["eyef"].ap(), G)
        tid1b = load("tid1b", [1, 1], F16, CO["eye1b"].ap(), S)
        tid1 = load("tid1", [1, 1], F32, CO["one1"].ap(), S)
        tonesr = load("tonesr", [1, 128], F32, CO["onesr"].ap(), S)

        masks = load("masks", [128, 15 * 128], F16, CO["masks"].ap(), G)
        aw1t_sb = load("aw1t_sb", [128, 8 * 128], F16, CO["aw1t"].ap(), G)
        aw2t_sb = load("aw2t_sb", [128, 9], F16, CO["aw2t"].ap(), G)
        cs_sb = load("cs_sb", [9, HW], F16, CO["cs"].ap(), G)
        paug = load("paug", [128, CT * 260], F16, CO["paug"].ap(), G)
        caw1t_sb = load("caw1t_sb", [128, 8 * 64], F16, CO["caw1t"].ap(), G)
        caw2t_sb = load("caw2t_sb", [64, C], F16, CO["caw2t"].ap(), G)
        spb = load("spb", [128, 6 * 128], F16, CO["spb"].ap(), G)
        g2c = load("g2c", [128, CT], F32, CO["g2c"].ap(), G)
        be2c = load("be2c", [128, CT], F32, CO["be2c"].ap(), G)
        g3r = load("g3r", [1, D], F32, CO["g3r"].ap(), G)
        be3r = load("be3r", [1, D], F32, CO["be3r"].ap(), G)
        sbc = load("sbc", [128, 1], F32, CO["sbc"].ap(), G)
        tonesrb = load("tonesrb", [1, 128], F16, CO["onesrb"].ap(), G)
        tonescb = load("tonescb", [128, 1], F16, CO["onescb"].ap(), G)
        ones8 = load("ones8", [8, 1], F32, CO["ones8"].ap(), G)

        # big working tensors
        yg = sb2.tile([128, CT * HW], F16, tag="yg")       # gelu1 out (ch-major)
        yg_sp = sb2.tile([128, HT * C], F16, tag="ygsp")   # transposed
        g2o = sb2.tile([128, CT * HW], F16, tag="yg")      # gelu2 out (reuses yg)
        yca = sb2.tile([128, CT * HW], F16, tag="ygsp")    # ch-att out (reuses)
        stat1l = sb.tile([128, 16], F32, tag="stat1l")
        stat1g = sb.tile([128, 16], F32, tag="stat1g")
        stat2l = sb.tile([128, 16], F32, tag="stat2l")
        stat2g = sb.tile([128, 16], F32, tag="stat2g")
        scr_v = sb.tile([128, HW], F32, tag="scr_v")        # DVE STT scratch
        scr_g = sb.tile([128, HW], F32, tag="scr_g")        # GPSIMD STT scratch
        dmy = sb.tile([1, 1], F32, tag="dmy")
        dmyo = sb.tile([1, 1], F32, tag="dmyo")
        nc.vector.memset(dmy[:], 1.0)
        epsc = sb.tile([128, 1], F32, tag="epsc")
        nc.vector.memset(epsc[:], EPS)

        # ============================ PHASE 1: expand + gelu1 + stats1
        for m in range(CT):
            ps = psB.tile([128, HW], F32, tag="psb")
            for h in range(2):
                for k in range(2):
                    nc.tensor.matmul(
                        ps[:, h * 512:(h + 1) * 512],
                        w1t_sb[:, k * C + m * 128: k * C + (m + 1) * 128],
                        xt_sb[:, k * HW + h * 512: k * HW + (h + 1) * 512],
                        start=(k == 0), stop=(k == 1))
            nc.scalar.activation(
                yg[:, m * HW:(m + 1) * HW], ps[:],
                AF.Gelu, bias=b1c[:, m:m + 1], scale=1.0,
                accum_out=stat1l[:, m:m + 1])
            srcg = yg[:, m * HW:(m + 1) * HW]
            nc.vector.scalar_tensor_tensor(
                scr_v[:], srcg, 0.0, srcg, OP.bypass, OP.mult,
                accum_out=stat1l[:, 8 + m:9 + m])

        # ---- transposes yg -> yg_sp (overlap AG1)
        for ht in range(HT):
            for grp in range(2):
                pt = psH.tile([128, 512], F16, tag="psh")
                for j in range(4):
                    cb = grp * 4 + j
                    nc.tensor.transpose(
                        pt[:, j * 128:(j + 1) * 128],
                        yg[:, cb * HW + ht * 128: cb * HW + (ht + 1) * 128],
                        tid[:])
                dst = yg_sp[:, ht * C + grp * 512: ht * C + (grp + 1) * 512]
                if grp == 0:
                    nc.vector.tensor_copy(dst, pt[:])
                else:
                    nc.scalar.copy(dst, pt[:])

        # ============================ AG1 (BN1 batch stats)
        bb1i = dram.tile([128, 16], F32, tag="bb1i")
        bb1o = dram.tile([n_cores * 128, 16], F32, tag="bb1o")
        nc.gpsimd.dma_start(bb1i[:], stat1l[:])
        nc.gpsimd.collective_compute(
            "AllGather", OP.bypass, replica_groups=[list(range(n_cores))],
            ins=[bb1i.opt()], outs=[bb1o.opt()])
        gath1 = sb.tile([128, n_cores * 16], F32, tag="gath1")
        nc.gpsimd.dma_start(
            gath1[:].rearrange("p (r f) -> p r f", f=16),
            bb1o[:].rearrange("(r p) f -> p r f", p=128))
        # PE p-state warmers: dummy transposes gated on the gathered stats,
        # so the PE is at speed when the stencil starts
        wps = psH.tile([128, 128], F32, tag="psh")
        for _ in range(20):
            nc.tensor.matmul(wps[:], gath1[:, 0:128], tidf[:],
                             is_transpose=True, skip_group_check=True)
        # combine 8 replicas in one strided reduce
        nc.vector.tensor_reduce(
            stat1g[:], gath1[:].rearrange("p (r f) -> p f r", f=16),
            mybir.AxisListType.X, OP.add)

        # ============================ BN1 affine + kw + L build
        def bn_affine(statg, gcol, becol, tagp):
            """-> (a, bn) per-channel scale/shift columns (128, CT), f32."""
            msq = sb.tile([128, 16], F32, tag=tagp + "m")
            var = sb.tile([128, CT], F32, tag=tagp + "v")
            sd = sb.tile([128, CT], F32, tag=tagp + "s")
            a = sb.tile([128, CT], F32, tag=tagp + "a")
            bn = sb.tile([128, CT], F32, tag=tagp + "b")
            nc.vector.tensor_scalar_mul(msq[:], statg[:], 1.0 / nb)
            mns, ex2 = msq[:, 0:8], msq[:, 8:16]
            nc.vector.tensor_tensor(var[:], mns, mns, OP.mult)
            nc.vector.tensor_tensor(var[:], ex2, var[:], OP.subtract)
            nc.scalar.activation(sd[:], var[:], AF.Sqrt, bias=epsc[:],
                                 scale=1.0)
            nc.vector.reciprocal(a[:], sd[:])
            nc.vector.tensor_tensor(a[:], gcol[:], a[:], OP.mult)
            nc.vector.tensor_tensor(bn[:], mns, a[:], OP.mult)
            nc.vector.tensor_tensor(bn[:], becol[:], bn[:], OP.subtract)
            return a, bn, msq

        a1, b1n, _ = bn_affine(stat1g, g1c, be1c, "s1")
        # b' = b1n / a1, as bf16 rows [CT, 128] (rank-1 lhsT slices)
        inva1 = sb.tile([128, CT], F32, tag="inva1")
        bpre = sb.tile([128, CT], F32, tag="bpre")
        nc.vector.reciprocal(inva1[:], a1[:])
        nc.vector.tensor_tensor(bpre[:], b1n[:], inva1[:], OP.mult)
        psb1 = psH.tile([CT, 128], F32, tag="psh")
        nc.tensor.transpose(psb1[:], bpre[:], tidf[:])
        b1pr8 = sb.tile([CT, 128], F16, tag="b1pr8")
        nc.vector.tensor_copy(b1pr8[:], psb1[:])
        # flatten [CT,128] -> [1, C] row via a DRAM bounce (engines cannot
        # move data across partitions; PE lhsT base partition must be 0)
        bpd = dram.tile([CT, 128], F16, tag="bpd")
        nc.gpsimd.dma_start(bpd[:], b1pr8[:])
        b1rowb = sb.tile([1, C], F16, tag="b1rowb")
        nc.gpsimd.dma_start(b1rowb[:], bpd[:].rearrange("c p -> (c p)")[None, :])

        # gap (local, normalized) -> kw
        gapnb = sb.tile([128, CT], F16, tag="gapnb")
        gapn = sb.tile([128, CT], F32, tag="gapn")
        nc.vector.scalar_tensor_tensor(gapn[:], stat1l[:, 0:8], 1.0 / HW,
                                       a1[:], OP.mult, OP.mult)
        nc.vector.tensor_tensor(gapnb[:], gapn[:], b1n[:], OP.add)

        ph1 = psH.tile([128, 1], F32, tag="psh")
        for k in range(CT):
            nc.tensor.matmul(ph1[:], aw1t_sb[:, k * 128:(k + 1) * 128],
                             gapnb[:, k:k + 1], start=(k == 0), stop=(k == 7))
        h1 = sb.tile([128, 1], F16, tag="h1")
        nc.scalar.activation(h1[:], ph1[:], AF.Relu, bias=ab1c[:], scale=1.0)
        ps9 = psH.tile([1, 9], F32, tag="psh")
        nc.tensor.matmul(ps9[:], h1[:], aw2t_sb[:], start=True, stop=True)
        v9 = sb.tile([1, 9], F32, tag="v9")
        nc.vector.tensor_tensor(v9[:], ps9[:], ab2r[:], OP.add)
        nmx = sb.tile([1, 1], F32, tag="nmx")
        nc.vector.tensor_reduce(nmx[:], v9[:], mybir.AxisListType.X, OP.max,
                                negate=True)
        e9 = sb.tile([1, 9], F32, tag="e9")
        se = sb.tile([1, 1], F32, tag="se")
        nc.scalar.activation(e9[:], v9[:], AF.Exp, bias=nmx[:], scale=1.0,
                             accum_out=se[:])
        rse = sb.tile([1, 1], F32, tag="rse")
        nc.vector.reciprocal(rse[:], se[:])
        kw9 = sb.tile([1, 9], F32, tag="kw9")
        nc.vector.tensor_scalar(kw9[:], e9[:], rse[:], None, OP.mult)

        # kw broadcast to all partitions (f32, for the DVE L build)
        pskb = psH.tile([128, 9], F32, tag="psh")
        nc.tensor.matmul(pskb[:], tonesr[:], kw9[:], start=True, stop=True)
        kwb = sb.tile([128, 9], F32, tag="kwb")
        nc.scalar.copy(kwb[:], pskb[:])
        # kw as bf16 column [9, 1] for the srow matmul
        pskc = psH.tile([9, 1], F32, tag="psh")
        nc.tensor.transpose(pskc[:], kw9[:], tid1[:])
        kwcb = sb.tile([9, 1], F16, tag="kwcb")
        nc.scalar.copy(kwcb[:], pskc[:])
        # srow = kw^T @ CS (tap-validity matrix), bf16
        srowb = sb.tile([1, HW], F16, tag="srowb")
        for h in range(2):
            psu = psH.tile([1, 512], F32, tag="psh")
            nc.tensor.matmul(psu[:], kwcb[:], cs_sb[:, h * 512:(h + 1) * 512],
                             start=True, stop=True)
            nc.scalar.copy(srowb[:, h * 512:(h + 1) * 512], psu[:])

        # L band tiles [L(-1) | L(0) | L(+1)]: ACT does the 6 side-block
        # products, DVE chains the 9 center STTs + 4 adds; bf16 at the end.
        L = sb.tile([128, 3 * 128], F32, tag="L")
        Lb = sb.tile([128, 3 * 128], F16, tag="Lb")
        prods = sb.tile([128, 6 * 128], F32, tag="prods")
        side_qs = [(0, 6), (1, 7), (2, 8), (12, 0), (13, 1), (14, 2)]
        for i, (blk, q) in enumerate(side_qs):
            nc.scalar.activation(prods[:, i * 128:(i + 1) * 128],
                                 masks[:, blk * 128:(blk + 1) * 128],
                                 AF.Copy, bias=0.0, scale=kwb[:, q:q + 1])
        nc.vector.tensor_scalar(L[:, 0:128], masks[:, 3 * 128:4 * 128],
                                kwb[:, 0:1], None, OP.mult)
        # Ld0 center chain: L[:,0:128] holds running sum of q0..q8 products
        for q in range(1, 8):
            nc.vector.scalar_tensor_tensor(
                L[:, 0:128], masks[:, (3 + q) * 128:(4 + q) * 128],
                kwb[:, q:q + 1], L[:, 0:128], OP.mult, OP.add)
        nc.vector.scalar_tensor_tensor(
            Lb[:, 128:256], masks[:, 11 * 128:12 * 128],
            kwb[:, 8:9], L[:, 0:128], OP.mult, OP.add)
        # Ldm = p0+p1+p2, Ldp = p3+p4+p5 (f32 partials from ACT, bf16 out)
        nc.vector.tensor_tensor(L[:, 128:256], prods[:, 0:128],
                                prods[:, 128:256], OP.add)
        nc.vector.tensor_tensor(Lb[:, 0:128], L[:, 128:256],
                                prods[:, 256:384], OP.add)
        nc.vector.tensor_tensor(L[:, 256:384], prods[:, 384:512],
                                prods[:, 512:640], OP.add)
        nc.vector.tensor_tensor(Lb[:, 256:384], L[:, 256:384],
                                prods[:, 640:768], OP.add)

        # ============================ PHASE 4: stencil + gelu2 + stats2
        for c in range(CT):
            psz = psB.tile([128, HW], F32, tag="psb")
            for h in range(2):
                nc.tensor.matmul(psz[:, h * 512:(h + 1) * 512],
                                 b1rowb[:, c * 128:(c + 1) * 128],
                                 srowb[:, h * 512:(h + 1) * 512],
                                 start=True, stop=False)
            for t_in in range(HT):
                lo = max(0, (t_in - 1) * 128)
                hi = min(HW, (t_in + 2) * 128)
                roff = 128 + (lo - t_in * 128)
                if lo < 512 < hi:
                    pieces = [(lo, 512), (512, hi)]
                else:
                    pieces = [(lo, hi)]
                for (a, b) in pieces:
                    ra = roff + (a - lo)
                    last_bank0 = (a < 512) and (t_in == 4)
                    last_bank1 = (a >= 512) and (t_in == 7)
                    nc.tensor.matmul(
                        psz[:, a:b],
                        yg_sp[:, t_in * C + c * 128: t_in * C + (c + 1) * 128],
                        Lb[:, ra:ra + (b - a)],
                        start=False, stop=(last_bank0 or last_bank1))
            nc.scalar.activation(
                g2o[:, c * HW:(c + 1) * HW], psz[:], AF.Gelu,
                bias=0.0, scale=a1[:, c:c + 1],
                accum_out=stat2l[:, c:c + 1])
            srcg2 = g2o[:, c * HW:(c + 1) * HW]
            nc.vector.scalar_tensor_tensor(
                scr_v[:], srcg2, 0.0, srcg2, OP.bypass, OP.mult,
                accum_out=stat2l[:, 8 + c:9 + c])

        # ============================ AG2 (BN2 batch stats)
        bb2i = dram.tile([128, 16], F32, tag="bb2i")
        bb2o = dram.tile([n_cores * 128, 16], F32, tag="bb2o")
        nc.gpsimd.dma_start(bb2i[:], stat2l[:])
        nc.gpsimd.collective_compute(
            "AllGather", OP.bypass, replica_groups=[list(range(n_cores))],
            ins=[bb2i.opt()], outs=[bb2o.opt()])
        gath2 = sb.tile([128, n_cores * 16], F32, tag="gath2")
        nc.gpsimd.dma_start(
            gath2[:].rearrange("p (r f) -> p r f", f=16),
            bb2o[:].rearrange("(r p) f -> p r f", p=128))
        wps2 = psH.tile([128, 128], F32, tag="psh")
        for _ in range(20):
            nc.tensor.matmul(wps2[:], gath2[:, 0:128], tidf[:],
                             is_transpose=True, skip_group_check=True)
        # overlap: per-channel max over HW of g2o (local)
        mxc = sb.tile([128, CT], F32, tag="mxc")
        for c in range(CT):
            nc.vector.tensor_reduce(mxc[:, c:c + 1],
                                    g2o[:, c * HW:(c + 1) * HW],
                                    mybir.AxisListType.X, OP.max)
        nc.vector.tensor_reduce(
            stat2g[:], gath2[:].rearrange("p (r f) -> p f r", f=16),
            mybir.AxisListType.X, OP.add)

        # ============================ BN2 + channel attention
        a2, b2n, _ = bn_affine(stat2g, g2c, be2c, "s2")
        amxb = sb.tile([128, 2 * CT], F16, tag="amxb")
        amx0 = sb.tile([128, CT], F32, tag="amx0")
        nc.vector.scalar_tensor_tensor(amx0[:], stat2l[:, 0:8], 1.0 / HW,
                                       a2[:], OP.mult, OP.mult)
        nc.vector.tensor_tensor(amxb[:, 0:8], amx0[:], b2n[:], OP.add)
        nc.vector.tensor_tensor(amx0[:], mxc[:], a2[:], OP.mult)
        nc.vector.tensor_tensor(amxb[:, 8:16], amx0[:], b2n[:], OP.add)

        psf = psH.tile([64, 2], F32, tag="psh")
        for k in range(CT):
            nc.tensor.matmul(psf[:], caw1t_sb[:, k * 64:(k + 1) * 64],
                             amxb[:, k:k + 9:8], start=(k == 0), stop=(k == 7))
        hp = sb.tile([64, 2], F16, tag="hp")
        nc.scalar.activation(hp[:], psf[:], AF.Relu, bias=0.0, scale=1.0)
        hsum = sb.tile([64, 1], F16, tag="hsum")
        nc.vector.tensor_tensor(hsum[:], hp[:, 0:1], hp[:, 1:2], OP.add)

        pss = psH.tile([128, CT], F32, tag="psh")
        for c in range(CT):
            nc.tensor.matmul(pss[:, c:c + 1], caw2t_sb[:, c * 128:(c + 1) * 128],
                             hsum[:], start=True, stop=True)
        scol = sb.tile([128, CT], F32, tag="scol")
        nc.scalar.activation(scol[:], pss[:], AF.Sigmoid, bias=0.0, scale=1.0)

        sprime = sb.tile([128, CT], F32, tag="sprime")
        b2s = sb.tile([128, CT], F32, tag="b2s")
        nc.vector.tensor_tensor(sprime[:], scol[:], a2[:], OP.mult)
        nc.vector.tensor_tensor(b2s[:], scol[:], b2n[:], OP.mult)
        b2sb = sb.tile([128, CT], F16, tag="b2sb")
        nc.vector.tensor_copy(b2sb[:], b2s[:])

        # y_ca (for the channel-max / channel-mean of spatial attention)
        for c in range(CT):
            nc.vector.tensor_scalar(yca[:, c * HW:(c + 1) * HW],
                                    g2o[:, c * HW:(c + 1) * HW],
                                    sprime[:, c:c + 1], b2s[:, c:c + 1],
                                    OP.mult, OP.add)

        # scaled projection weights
        pws = sb2.tile([128, CT * 260], F16, tag="pws")
        for c in range(CT):
            nc.vector.tensor_scalar(pws[:, c * 260:c * 260 + 258],
                                    paug[:, c * 260:c * 260 + 258],
                                    sprime[:, c:c + 1], None, OP.mult)
        # t2 row (rank-1 bias of the projection)
        pst2 = psH.tile([1, 258], F32, tag="psh")
        for c in range(CT):
            nc.tensor.matmul(pst2[:], b2sb[:, c:c + 1],
                             paug[:, c * 260:c * 260 + 258],
                             start=(c == 0), stop=(c == 7))
        u2row = sb.tile([1, 258], F16, tag="u2row")
        nc.scalar.copy(u2row[:], pst2[:])

        # projection -> proj_sb (spatial-major (hw, d)), bf16
        proj_sb = sb2.tile([128, 8 * D], F16, tag="proj_sb")
        avgpx = sb.tile([128, HT], F16, tag="avgpx")
        for mt in range(HT):
            psp = psH.tile([128, 258], F32, tag="psh")
            for c in range(CT):
                nc.tensor.matmul(psp[:],
                                 g2o[:, c * HW + mt * 128: c * HW + (mt + 1) * 128],
                                 pws[:, c * 260:c * 260 + 258],
                                 start=(c == 0), stop=False)
            nc.tensor.matmul(psp[:], tonesrb[:], u2row[:], start=False,
                             stop=True)
            dst = proj_sb[:, mt * D:(mt + 1) * D]
            if mt % 2 == 0:
                nc.vector.tensor_copy(dst, psp[:, 0:D])
            else:
                nc.scalar.copy(dst, psp[:, 0:D])
            nc.vector.tensor_copy(avgpx[:, mt:mt + 1], psp[:, D:D + 1])

        # channel max per pixel: tree over c-tiles (destroys yca), then
        # PE-transpose each 128-pixel block and DVE-reduce over channels
        for i in range(4):
            nc.vector.tensor_tensor(yca[:, (2 * i) * HW:(2 * i + 1) * HW],
                                    yca[:, (2 * i) * HW:(2 * i + 1) * HW],
                                    yca[:, (2 * i + 1) * HW:(2 * i + 2) * HW],
                                    OP.max)
        nc.vector.tensor_tensor(yca[:, 0:HW], yca[:, 0:HW],
                                yca[:, 2 * HW:3 * HW], OP.max)
        nc.vector.tensor_tensor(yca[:, 4 * HW:5 * HW], yca[:, 4 * HW:5 * HW],
                                yca[:, 6 * HW:7 * HW], OP.max)
        nc.vector.tensor_tensor(yca[:, 0:HW], yca[:, 0:HW],
                                yca[:, 4 * HW:5 * HW], OP.max)
        mxpx = sb.tile([128, HT], F16, tag="mxpx")
        for t in range(HT):
            pmx = psH.tile([128, 128], F16, tag="psh")
            nc.tensor.transpose(pmx[:], yca[:, t * 128:(t + 1) * 128], tid[:])
            nc.vector.tensor_reduce(mxpx[:, t:t + 1], pmx[:],
                                    mybir.AxisListType.X, OP.max)

        # spatial 7x7 conv as 6 shifted-column matmuls, one sigmoid
        pssp = psH.tile([128, HT], F32, tag="psh")
        mmspecs = []
        for ch, srccol in ((0, avgpx), (1, mxpx)):
            mmspecs.append((ch * 3 + 1, slice(0, 8), srccol[:, 0:8]))
            mmspecs.append((ch * 3 + 2, slice(1, 8), srccol[:, 0:7]))
            mmspecs.append((ch * 3 + 0, slice(0, 7), srccol[:, 1:8]))
        for i, (bi, osl, rhs) in enumerate(mmspecs):
            nc.tensor.matmul(pssp[:, osl], spb[:, bi * 128:(bi + 1) * 128],
                             rhs, start=(i == 0), stop=(i == len(mmspecs) - 1))
        spcol = sb.tile([128, HT], F32, tag="spcol")
        nc.scalar.activation(spcol[:], pssp[:], AF.Sigmoid, bias=sbc[:],
                             scale=1.0)
        spcolb = sb.tile([128, HT], F16, tag="spcolb")
        nc.vector.tensor_copy(spcolb[:], spcol[:])

        # spp = proj * sp (spatial scale, per-partition)
        spp = sb2.tile([128, 8 * D], F16, tag="spp")
        for mt in range(HT):
            nc.scalar.activation(spp[:, mt * D:(mt + 1) * D],
                                 proj_sb[:, mt * D:(mt + 1) * D],
                                 AF.Copy, bias=0.0,
                                 scale=spcol[:, mt:mt + 1])

        # BN3 stats: sum(sp*proj) and sum((sp*proj)^2) over hw
        pst3a = psH.tile([1, D], F32, tag="psh")
        for mt in range(HT):
            nc.tensor.matmul(pst3a[:], spcolb[:, mt:mt + 1],
                             proj_sb[:, mt * D:(mt + 1) * D],
                             start=(mt == 0), stop=(mt == 7))
        pst3b = psH.tile([1, D], F32, tag="psh")
        sqs = sb.tile([128, 2 * D], F16, tag="sqs")
        for mt in range(HT):
            half = (mt % 2) * D
            nc.scalar.square(sqs[:, half:half + D], spp[:, mt * D:(mt + 1) * D])
            nc.tensor.matmul(pst3b[:], tonescb[:], sqs[:, half:half + D],
                             start=(mt == 0), stop=(mt == 7))
        stat3l = sb.tile([1, 2 * D], F32, tag="stat3l")
        nc.scalar.copy(stat3l[:, 0:D], pst3a[:])
        nc.vector.tensor_copy(stat3l[:, D:2 * D], pst3b[:])

        # ============================ AG3 (BN3 batch stats)
        bb3i = dram.tile([1, 2 * D], F32, tag="bb3i")
        bb3o = dram.tile([n_cores, 2 * D], F32, tag="bb3o")
        nc.gpsimd.dma_start(bb3i[:], stat3l[:])
        nc.gpsimd.collective_compute(
            "AllGather", OP.bypass, replica_groups=[list(range(n_cores))],
            ins=[bb3i.opt()], outs=[bb3o.opt()])
        gath3p = sb.tile([8, 2 * D], F32, tag="gath3p")
        nc.gpsimd.dma_start(gath3p[:], bb3o[:])
        # PE p-state warmers (read-only on gath3p -> no WAR stall)
        wps3 = psH.tile([128, 8], F32, tag="psh")
        for _ in range(12):
            nc.tensor.matmul(wps3[:], gath3p[:, 0:128], tidf[0:8, 0:8],
                             is_transpose=True, skip_group_check=True)
        # combine the 8 replicas with one K=8 matmul
        pst3g = psH.tile([1, 2 * D], F32, tag="psh")
        nc.tensor.matmul(pst3g[:], ones8[:], gath3p[:], start=True, stop=True)

        # BN3 affine broadcast-first: one row op, then everything at
        # [128, D] full-lane width (pb cancels against the mean)
        msq3 = sb.tile([1, 2 * D], F16, tag="msq3")
        nc.vector.tensor_scalar(msq3[:], pst3g[:], 1.0 / nb, None, OP.mult)
        m3b = sb.tile([128, D], F32, tag="m3b")
        psx = psH.tile([128, 2 * D], F32, tag="psh")
        nc.tensor.matmul(psx[:, 0:D], tonesrb[:], msq3[:, 0:D],
                         start=True, stop=True)
        nc.tensor.matmul(psx[:, D:2 * D], tonesrb[:], msq3[:, D:2 * D],
                         start=True, stop=True)
        nc.vector.tensor_copy(m3b[:], psx[:, 0:D])
        vb = sb.tile([128, D], F32, tag="vb")
        nc.vector.tensor_tensor(vb[:], m3b[:], m3b[:], OP.mult)
        nc.vector.tensor_tensor(vb[:], psx[:, D:2 * D], vb[:], OP.subtract)
        nc.scalar.activation(vb[:], vb[:], AF.Sqrt, bias=epsc[:], scale=1.0)
        a3b = sb.tile([128, D], F16, tag="a3b")
        c3b = sb.tile([128, D], F16, tag="c3b")
        rb = sb.tile([128, D], F32, tag="rb")
        nc.vector.reciprocal(rb[:], vb[:])
        nc.vector.tensor_tensor(a3b[:], g3b[:], rb[:], OP.mult)
        nc.vector.tensor_tensor(rb[:], m3b[:], rb[:], OP.mult)
        nc.vector.tensor_tensor(rb[:], g3b[:], rb[:], OP.mult)
        nc.vector.tensor_tensor(c3b[:], be3b[:], rb[:], OP.subtract)

        # final: out = (xres + c3) + spp*a3
        # gpsimd computes xres+c3 per block; DVE does the mult and final add
        xc = sb2.tile([128, 8 * D], F32, tag="xc")
        out_sb = sb2.tile([128, 8 * D], F32, tag="outsb")
        tmp = sb.tile([128, D], F16, tag="ftmp")
        for mt in range(HT):
            sl = slice(mt * D, (mt + 1) * D)
            nc.gpsimd.tensor_tensor(xc[:, sl], xres[:, sl], c3b[:], OP.add)
            nc.vector.tensor_tensor(tmp[:], spp[:, sl], a3b[:], OP.mult)
            nc.vector.tensor_tensor(out_sb[:, sl], xc[:, sl], tmp[:], OP.add)
            q = nc.sync if mt % 2 == 0 else nc.scalar
            q.dma_start(out_d.ap()[:, mt * D:(mt + 1) * D], out_sb[:, sl])


# ---------------------------------------------------------------- host driver

def shard_inputs(inputs):
    x = np.ascontiguousarray(np.asarray(inputs["x"], np.float32))
    in_maps = []
    for i in range(NCORES):
        in_maps.append({
            "xt": _interleave(x[i].T, 2).astype(BF),
            "xres": _interleave(x[i], 8).astype(BF),
        })
    return in_maps


_CACHE = {}


def get_program(inputs, n_cores=NCORES):
    W = prep_weights(inputs)
    h = hashlib.sha256()
    for k in sorted(W):
        h.update(k.encode())
        h.update(np.ascontiguousarray(W[k]).tobytes())
    key = (n_cores, h.hexdigest())
    if key not in _CACHE:
        _CACHE[key] = build_program(W, n_cores=n_cores)
    return _CACHE[key]


def run(inputs, trace=False):
    nc = get_program(inputs)
    in_maps = shard_inputs(inputs)
    r = bass_utils.run_bass_kernel_spmd(
        nc, in_maps, core_ids=list(range(NCORES)), trace=trace)
    out = np.stack(
        [r.results[i]["out"].reshape(128, 8, D).transpose(1, 0, 2)
         .reshape(HW, D) for i in range(NCORES)], axis=0)
    return np.ascontiguousarray(out.astype(np.float32)), r


def kernel(**inputs) -> np.ndarray:
    out, _ = run(inputs, trace=False)
    return out
